# revision 3
# baseline (speedup 1.0000x reference)
"""Single-head causal attention (B=8, T=2048, C=256, H=64) on 8 TRN2 NeuronCores.

Sharding: batch dim across the 8 cores (data parallel, one batch element per
core); each core computes its full TxT causal attention independently.

Per-core algorithm (x_b = x[b], shape [T, C]):
  qT = Wq.T @ x_b.T        [H, T]   (host supplies x_b.T; [Wq|Wk] packed so one
  kT = Wk.T @ x_b.T        [H, T]    matmul stream yields [qT; kT] stacked)
  weiT[tk, tq] = kT.T-chunk @ qT  (so the PV matmul needs no transpose)
  e = exp(0.125 * weiT) with causal mask (additive -1e30 on diagonal blocks,
      fully-masked tile pairs skipped entirely)
  outT_aug = v_aug.T @ e   where v_aug = [v; ones] -> row 64 = softmax rowsum
  out = (outT_aug[:64] / outT_aug[64]).T   (PE transpose + per-row reciprocal)

All matmuls run in float32r (1 cycle/row on TRN2 vs 4 for fp32; ~1.7e-4
relative rounding), fp32 everywhere else.
"""

import numpy as np

import concourse.bass as bass
import concourse.mybir as mybir
import concourse.tile as tile
from concourse import bass_utils

B, T, C, H = 8, 2048, 256, 64
NCC = C // 128          # 2 c-chunks
NTQ = T // 512          # 4 tq blocks
NTK = T // 128          # 16 tk chunks
NEG = -1.0e30

dt = mybir.dt
MM_DT = dt.float32r     # matmul operand dtype; dt.float32 for full precision
F32 = dt.float32


def _split_excess_waits(nc, max_waits=1):
    """The walrus build in this container rejects >1 sync wait per
    instruction ("Too many sync wait commands"); spill extras onto
    preceding same-engine NoOps (same AND semantics, engine blocks at the
    NoOp until the semaphore condition holds)."""
    for f in nc.m.functions:
        for bb in f.blocks:
            new = []
            for inst in bb.instructions:
                si = inst.sync_info
                waits = list(si.on_wait) if si is not None else []
                if len(waits) > max_waits:
                    extra, keep = waits[:-max_waits], waits[-max_waits:]
                    for i in range(0, len(extra), max_waits):
                        chunk = extra[i:i + max_waits]
                        nop = mybir.InstNoOp(
                            name=nc.get_next_instruction_name(),
                            engine=inst.engine,
                            ins=[], outs=[],
                            sync_info=mybir.SyncInfo(on_wait=chunk, on_update=[]),
                        )
                        nc.register_instruction(nop)
                        new.append(nop)
                    inst.sync_info = mybir.SyncInfo(
                        on_wait=keep, on_update=list(si.on_update))
                new.append(inst)
            bb.instructions = new


def _patch_tile_drain():
    """Tile's kernel-tail drain carries one wait per live semaphore; split
    them the same way (idempotent monkeypatch)."""
    from concourse.vector_clock import ScopedClock

    if getattr(tile.TileContext, "_ant_drain_patched", False):
        return

    def _drain_and_barrier(self, tick_clock, wait_clock):
        drain_inst = self.nc.sync.drain()
        wait_clock.add_sem_waits(
            drain_inst.ins, ScopedClock({None: tick_clock.global_clock}))
        si = drain_inst.ins.sync_info
        waits = list(si.on_wait) if si is not None else []
        if len(waits) > 1:
            drain_inst.ins.sync_info = mybir.SyncInfo(
                on_wait=[waits[0]], on_update=list(si.on_update))
            for w in waits[1:]:
                ni = self.nc.sync.nop(nofuse=True)
                ni.ins.sync_info = mybir.SyncInfo(on_wait=[w], on_update=[])
        self.nc.all_engine_barrier()
        assert self.sems is not None
        popped = self.nc._tile_sem_poison_stack.pop()
        assert popped is self._sem_poison
        self.nc.clear_and_free_semaphores(list(self.sems.allocated().values()))
        self.nc.all_engine_barrier()

    tile.TileContext._drain_and_barrier = _drain_and_barrier
    tile.TileContext._ant_drain_patched = True


def _attention_body(nc, tc, pools, dram):
    """Emit one pass of the per-core attention computation."""
    persist, epool, onat, psw, pso, pst = pools
    xt_d, wa_d, wv_d, tri_d, idn_d, out_d = dram
    Exp = mybir.ActivationFunctionType.Exp
    Copy = mybir.ActivationFunctionType.Copy

    # ---- persistent SBUF tensors -------------------------------------
    xt = persist.tile([128, NCC, T], MM_DT, tag="xt")
    wa = persist.tile([128, NCC, 128], MM_DT, tag="wa")
    wv = persist.tile([128, NCC, H], MM_DT, tag="wv")
    tri = persist.tile([128, 128], F32, tag="tri")
    idn = persist.tile([128, 128], F32, tag="idn")
    proj_a = persist.tile([128, T], MM_DT, tag="proj_a")   # [qT; kT]
    proj_b = persist.tile([128, T], MM_DT, tag="proj_b")   # [kT; qT]
    vt = persist.tile([H + 1, T], F32, tag="vt")           # vT + ones row
    vaug = persist.tile([128, NTK, H + 1], MM_DT, tag="vaug")

    for cc in range(NCC):
        nc.sync.dma_start(wa[:, cc, :], wa_d[cc])
        nc.sync.dma_start(wv[:, cc, :], wv_d[cc])
    nc.sync.dma_start(tri[:], tri_d[:])
    nc.sync.dma_start(idn[:], idn_d[:])
    for j in range(NTQ):
        for cc in range(NCC):
            nc.sync.dma_start(xt[:, cc, 512 * j:512 * (j + 1)],
                              xt_d[cc, :, 512 * j:512 * (j + 1)])

    # ---- projections: proj_a = [Wq|Wk].T @ xT, vT = Wv.T @ xT --------
    for j in range(NTQ):
        sl = slice(512 * j, 512 * (j + 1))
        pp = psw.tile([128, 1024], F32, tag="w")
        for cc in range(NCC):
            nc.tensor.matmul(pp[:, 0:512], wa[:, cc, :], xt[:, cc, sl],
                             start=(cc == 0), stop=(cc == NCC - 1))
        eng = nc.vector if j % 2 == 0 else nc.scalar
        if eng is nc.vector:
            nc.vector.tensor_copy(proj_a[:, sl], pp[:, 0:512])
        else:
            nc.scalar.copy(proj_a[:, sl], pp[:, 0:512])

        pv = psw.tile([128, 1024], F32, tag="w")
        for cc in range(NCC):
            nc.tensor.matmul(pv[0:H, 0:512], wv[:, cc, :], xt[:, cc, sl],
                             start=(cc == 0), stop=(cc == NCC - 1))
        if eng is nc.vector:
            nc.scalar.copy(vt[0:H, sl], pv[0:H, 0:512])
        else:
            nc.vector.tensor_copy(vt[0:H, sl], pv[0:H, 0:512])

    # proj_b = [kT; qT] by swapping the halves of proj_a (SBUF->SBUF DMA)
    nc.sync.dma_start(proj_b[0:64, :], proj_a[64:128, :])
    nc.sync.dma_start(proj_b[64:128, :], proj_a[0:64, :])

    # ones row for the rowsum trick
    nc.gpsimd.memset(vt[H:H + 1, :], 1.0)

    # ---- v_aug[i] = (vT_aug[:, 128i:128(i+1)]).T  via PE transpose ----
    for g in range(NTK // 4):
        pt = pst.tile([128, 4, H + 1], F32, tag="t")
        for t in range(4):
            i = 4 * g + t
            nc.tensor.transpose(pt[:, t, :], vt[:, 128 * i:128 * (i + 1)],
                                idn[0:H + 1, 0:H + 1])
        nc.vector.tensor_copy(vaug[:, 4 * g:4 * g + 4, :], pt[:])

    # ---- main loop over tq blocks ------------------------------------
    for j in range(NTQ):
        sl = slice(512 * j, 512 * (j + 1))
        nk = 4 * j + 4                      # valid tk chunks (causal)
        po = pso.tile([H + 1, 512], F32, tag="o")
        for p in range(nk // 2):
            i0, i1 = 2 * p, 2 * p + 1
            wp = psw.tile([128, 1024], F32, tag="w")
            # dual row-group QK: even chunk on PE rows 0-63, odd on 64-127
            nc.tensor.matmul(wp[:, 0:512],
                             proj_b[0:64, 128 * i0:128 * (i0 + 1)],
                             proj_a[0:64, sl], start=True, stop=True)
            nc.tensor.matmul(wp[:, 512:1024],
                             proj_a[64:128, 128 * i1:128 * (i1 + 1)],
                             proj_b[64:128, sl], start=True, stop=True,
                             tile_position=(64, 0))
            e = epool.tile([128, 1024], MM_DT, tag="e")
            d0, d1 = i0 - 4 * j, i1 - 4 * j
            if d1 < 0:          # both tiles fully unmasked
                nc.scalar.activation(e[:], wp[:], Exp, scale=0.125)
            else:
                for off, i, d in ((0, i0, d0), (512, i1, d1)):
                    # additive tri mask on the diagonal 128-col strip
                    nc.vector.tensor_add(
                        wp[:, off + 128 * d:off + 128 * (d + 1)],
                        wp[:, off + 128 * d:off + 128 * (d + 1)], tri[:])
                    nc.scalar.activation(
                        e[:, off + 128 * d:off + 512],
                        wp[:, off + 128 * d:off + 512], Exp, scale=0.125)
                    if d > 0:   # cols left of the strip are fully masked
                        nc.gpsimd.memset(
                            e[:, off:off + 128 * d].bitcast(F32), 0.0)
            nc.tensor.matmul(po[:], vaug[:, i0, :], e[:, 0:512],
                             start=(i0 == 0), stop=False)
            nc.tensor.matmul(po[:], vaug[:, i1, :], e[:, 512:1024],
                             start=False, stop=(i1 == nk - 1))

        # epilogue: transpose outT_aug back, normalize, store
        ot = onat.tile([H + 1, 512], F32, tag="ot")
        nc.scalar.copy(ot[:], po[:])
        pt = pst.tile([128, 4, H + 1], F32, tag="t")
        for t in range(4):
            nc.tensor.transpose(pt[:, t, :], ot[:, 128 * t:128 * (t + 1)],
                                idn[0:H + 1, 0:H + 1], )
        on = onat.tile([128, 4, H], F32, tag="on")
        for t in range(4):
            rc = onat.tile([128, 1], F32, tag="rc")
            nc.vector.reciprocal(rc[:], pt[:, t, H:H + 1])
            nc.scalar.activation(on[:, t, :], pt[:, t, 0:H], Copy,
                                 scale=rc[:, 0:1])
        dst = out_d[sl].rearrange("(t p) h -> p t h", p=128)
        nc.sync.dma_start(dst, on[:])


def build_nc(repeats=1, mm_dt=None):
    """Build the per-core Bass program (SPMD: same program on all 8 cores).

    repeats > 1 wraps the body in an on-device For_i loop; used only by the
    benchmarking harness to amortize host/launch overhead out of timing.
    """
    global MM_DT
    if mm_dt is not None:
        MM_DT = mm_dt
    _patch_tile_drain()
    nc = bass.Bass("TRN2", target_bir_lowering=False, debug=False)

    xt_d = nc.dram_tensor("xt", [NCC, 128, T], MM_DT, kind="ExternalInput")
    wa_d = nc.dram_tensor("wa", [NCC, 128, 128], MM_DT, kind="ExternalInput")
    wv_d = nc.dram_tensor("wv", [NCC, 128, H], MM_DT, kind="ExternalInput")
    tri_d = nc.dram_tensor("tri", [128, 128], F32, kind="ExternalInput")
    idn_d = nc.dram_tensor("idn", [128, 128], F32, kind="ExternalInput")
    out_d = nc.dram_tensor("out", [T, H], F32, kind="ExternalOutput")
    dram = (xt_d, wa_d, wv_d, tri_d, idn_d, out_d)

    with tile.TileContext(nc) as tc:
        with (
            tc.tile_pool(name="persist", bufs=1) as persist,
            tc.tile_pool(name="epool", bufs=3) as epool,
            tc.tile_pool(name="onat", bufs=2) as onat,
            tc.tile_pool(name="psw", bufs=2, space="PSUM") as psw,
            tc.tile_pool(name="pso", bufs=2, space="PSUM") as pso,
            tc.tile_pool(name="pst", bufs=2, space="PSUM") as pst,
        ):
            pools = (persist, epool, onat, psw, pso, pst)
            if repeats == 1:
                _attention_body(nc, tc, pools, dram)
            else:
                with tc.For_i(0, repeats, 1):
                    _attention_body(nc, tc, pools, dram)
    _split_excess_waits(nc)
    return nc


def make_in_maps(x, Wk, Wq, Wv):
    """Host-side layout prep: per-core transposed x, packed weights, masks."""
    x = np.asarray(x, dtype=np.float32)
    Wk = np.asarray(Wk, dtype=np.float32)
    Wq = np.asarray(Wq, dtype=np.float32)
    Wv = np.asarray(Wv, dtype=np.float32)

    wa = np.concatenate([Wq, Wk], axis=1).reshape(NCC, 128, 128)
    wv = Wv.reshape(NCC, 128, H)
    r = np.arange(128)
    tri = np.where(r[:, None] <= r[None, :], 0.0, NEG).astype(np.float32)
    idn = np.eye(128, dtype=np.float32)
    common = {"wa": np.ascontiguousarray(wa), "wv": np.ascontiguousarray(wv),
              "tri": tri, "idn": idn}
    in_maps = []
    for b in range(B):
        xt = np.ascontiguousarray(x[b].T).reshape(NCC, 128, T)
        in_maps.append({"xt": xt, **common})
    return in_maps


def kernel(x, Wk, Wq, Wv):
    nc = build_nc(repeats=1)
    in_maps = make_in_maps(x, Wk, Wq, Wv)
    res = bass_utils.run_bass_kernel_spmd(nc, in_maps, core_ids=list(range(B)))
    return np.stack([res.results[b]["out"] for b in range(B)], axis=0)


# revision 10
# speedup vs baseline: 2.0230x; 2.0230x over previous
"""Single-head causal attention (B=8, T=2048, C=256, H=64) on 8 TRN2 NeuronCores.

Sharding: batch dim across the 8 cores (data parallel, one batch element per
core); each core computes its full TxT causal attention independently.

Per-core algorithm (x_b = x[b], shape [T, C]):
  qT = Wq.T @ x_b.T        [H, T]   (host supplies x_b.T; [Wq|Wk] packed so one
  kT = Wk.T @ x_b.T        [H, T]    matmul stream yields [qT; kT] stacked)
  weiT[tk, tq] = kT.T-chunk @ qT  (so the PV matmul needs no transpose)
  e = exp(0.125 * weiT) with causal mask (additive -1e30 on diagonal blocks,
      fully-masked tile pairs skipped entirely)
  outT_aug = v_aug.T @ e   where v_aug = [v; ones] -> row 64 = softmax rowsum
  out = (outT_aug[:64] / outT_aug[64]).T   (PE transpose + per-row reciprocal)

All matmuls run in float32r (1 cycle/row on TRN2 vs 4 for fp32; ~1.7e-4
relative rounding), fp32 everywhere else.
"""

import numpy as np

import concourse.bass as bass
import concourse.mybir as mybir
import concourse.tile as tile
from concourse import bass_utils

B, T, C, H = 8, 2048, 256, 64
NCC = C // 128          # 2 c-chunks
NTQ = T // 512          # 4 tq blocks
NTK = T // 128          # 16 tk chunks
NEG = -1.0e30

dt = mybir.dt
MM_DT = dt.float32r     # matmul operand dtype; dt.float32 for full precision
F32 = dt.float32


def _split_excess_waits(nc, max_waits=1):
    """The walrus build in this container rejects >1 sync wait per
    instruction ("Too many sync wait commands"); spill extras onto
    preceding same-engine NoOps (same AND semantics, engine blocks at the
    NoOp until the semaphore condition holds)."""
    for f in nc.m.functions:
        for bb in f.blocks:
            new = []
            for inst in bb.instructions:
                si = inst.sync_info
                waits = list(si.on_wait) if si is not None else []
                if len(waits) > max_waits:
                    extra, keep = waits[:-max_waits], waits[-max_waits:]
                    for i in range(0, len(extra), max_waits):
                        chunk = extra[i:i + max_waits]
                        nop = mybir.InstNoOp(
                            name=nc.get_next_instruction_name(),
                            engine=inst.engine,
                            ins=[], outs=[],
                            sync_info=mybir.SyncInfo(on_wait=chunk, on_update=[]),
                        )
                        nc.register_instruction(nop)
                        new.append(nop)
                    inst.sync_info = mybir.SyncInfo(
                        on_wait=keep, on_update=list(si.on_update))
                new.append(inst)
            bb.instructions = new


def _patch_tile_drain():
    """Tile's kernel-tail drain carries one wait per live semaphore; split
    them the same way (idempotent monkeypatch)."""
    from concourse.vector_clock import ScopedClock

    if getattr(tile.TileContext, "_ant_drain_patched", False):
        return

    def _drain_and_barrier(self, tick_clock, wait_clock):
        drain_inst = self.nc.sync.drain()
        wait_clock.add_sem_waits(
            drain_inst.ins, ScopedClock({None: tick_clock.global_clock}))
        si = drain_inst.ins.sync_info
        waits = list(si.on_wait) if si is not None else []
        if len(waits) > 1:
            drain_inst.ins.sync_info = mybir.SyncInfo(
                on_wait=[waits[0]], on_update=list(si.on_update))
            for w in waits[1:]:
                ni = self.nc.sync.nop(nofuse=True)
                ni.ins.sync_info = mybir.SyncInfo(on_wait=[w], on_update=[])
        self.nc.all_engine_barrier()
        assert self.sems is not None
        popped = self.nc._tile_sem_poison_stack.pop()
        assert popped is self._sem_poison
        self.nc.clear_and_free_semaphores(list(self.sems.allocated().values()))
        self.nc.all_engine_barrier()

    tile.TileContext._drain_and_barrier = _drain_and_barrier
    tile.TileContext._ant_drain_patched = True


def _attention_body(nc, tc, pools, dram):
    """Emit one pass of the per-core attention computation."""
    persist, epool, onat, psw, pso, pst = pools
    xt_d, wa_d, wv_d, tri_d, idn_d, out_d = dram
    Exp = mybir.ActivationFunctionType.Exp
    Copy = mybir.ActivationFunctionType.Copy

    # ---- persistent SBUF tensors -------------------------------------
    xt = persist.tile([128, NCC, T], MM_DT, tag="xt")
    wa = persist.tile([128, NCC, 128], MM_DT, tag="wa")
    wv = persist.tile([128, NCC, H], MM_DT, tag="wv")
    tri = persist.tile([128, 128], F32, tag="tri")
    idn = persist.tile([128, 128], F32, tag="idn")
    proj_a = persist.tile([128, T], MM_DT, tag="proj_a")   # [qT; kT]
    proj_b = persist.tile([128, T], MM_DT, tag="proj_b")   # [kT; qT]
    vt = persist.tile([H + 1, T], F32, tag="vt")           # vT + ones row
    vaug = persist.tile([128, NTK, H + 1], MM_DT, tag="vaug")

    for cc in range(NCC):
        nc.scalar.dma_start(wa[:, cc, :], wa_d[cc])
        nc.scalar.dma_start(wv[:, cc, :], wv_d[cc])
    nc.scalar.dma_start(tri[:], tri_d[:])
    nc.scalar.dma_start(idn[:], idn_d[:])
    for j in range(NTQ):
        for cc in range(NCC):
            # split the 2 MB x-load across the two HWDGE issuers (SP/ACT)
            eng = nc.sync if (j + cc) % 2 == 0 else nc.scalar
            eng.dma_start(xt[:, cc, 512 * j:512 * (j + 1)],
                          xt_d[cc, :, 512 * j:512 * (j + 1)])

    # ---- projections: proj_a = [Wq|Wk].T @ xT, vT = Wv.T @ xT --------
    for j in range(NTQ):
        sl = slice(512 * j, 512 * (j + 1))
        pp = psw.tile([128, 1024], F32, tag="w")
        for cc in range(NCC):
            nc.tensor.matmul(pp[:, 0:512], wa[:, cc, :], xt[:, cc, sl],
                             start=(cc == 0), stop=(cc == NCC - 1))
        nc.vector.tensor_copy(proj_a[:, sl], pp[:, 0:512])

        pv = psw.tile([128, 1024], F32, tag="w")
        for cc in range(NCC):
            nc.tensor.matmul(pv[0:H, 0:512], wv[:, cc, :], xt[:, cc, sl],
                             start=(cc == 0), stop=(cc == NCC - 1))
        nc.vector.tensor_copy(vt[0:H, sl], pv[0:H, 0:512])

    # proj_b = [kT; qT] by swapping the halves of proj_a (SBUF->SBUF DMA)
    nc.sync.dma_start(proj_b[0:64, :], proj_a[64:128, :])
    nc.sync.dma_start(proj_b[64:128, :], proj_a[0:64, :])

    # ones row for the rowsum trick
    nc.gpsimd.memset(vt[H:H + 1, :], 1.0)

    # ---- v_aug[i] = (vT_aug[:, 128i:128(i+1)]).T  via PE transpose ----
    for g in range(NTK // 4):
        pt = pst.tile([128, 4, H + 1], F32, tag="t")
        for t in range(4):
            i = 4 * g + t
            nc.tensor.transpose(pt[:, t, :], vt[:, 128 * i:128 * (i + 1)],
                                idn[0:H + 1, 0:H + 1])
        nc.vector.tensor_copy(vaug[:, 4 * g:4 * g + 4, :], pt[:])

    # ---- main loop over tq blocks ------------------------------------
    for j in range(NTQ):
        sl = slice(512 * j, 512 * (j + 1))
        nk = 4 * j + 4                      # valid tk chunks (causal)
        po = pso.tile([H + 1, 512], F32, tag="o")
        for p in range(nk // 2):
            i0, i1 = 2 * p, 2 * p + 1
            wp = psw.tile([128, 1024], F32, tag="w")
            # dual row-group QK: even chunk on PE rows 0-63, odd on 64-127
            nc.tensor.matmul(wp[:, 0:512],
                             proj_b[0:64, 128 * i0:128 * (i0 + 1)],
                             proj_a[0:64, sl], start=True, stop=True)
            nc.tensor.matmul(wp[:, 512:1024],
                             proj_a[64:128, 128 * i1:128 * (i1 + 1)],
                             proj_b[64:128, sl], start=True, stop=True,
                             tile_position=(64, 0))
            e = epool.tile([128, 1024], MM_DT, tag="e")
            d0, d1 = i0 - 4 * j, i1 - 4 * j
            if d1 < 0:          # both tiles fully unmasked
                nc.scalar.activation(e[:], wp[:], Exp, scale=0.125)
            else:
                for off, i, d in ((0, i0, d0), (512, i1, d1)):
                    nc.scalar.activation(
                        e[:, off + 128 * d:off + 512],
                        wp[:, off + 128 * d:off + 512], Exp, scale=0.125)
                    # multiplicative tri mask on the diagonal 128-col strip
                    nc.vector.tensor_mul(
                        e[:, off + 128 * d:off + 128 * (d + 1)],
                        e[:, off + 128 * d:off + 128 * (d + 1)], tri[:])
                    if d > 0:   # cols left of the strip are fully masked
                        nc.gpsimd.memset(
                            e[:, off:off + 128 * d].bitcast(F32), 0.0)
            nc.tensor.matmul(po[:], vaug[:, i0, :], e[:, 0:512],
                             start=(i0 == 0), stop=False)
            nc.tensor.matmul(po[:], vaug[:, i1, :], e[:, 512:1024],
                             start=False, stop=(i1 == nk - 1))

        # epilogue: transpose outT_aug back, normalize, store
        ot = onat.tile([H + 1, 512], F32, tag="ot")
        nc.vector.tensor_copy(ot[:], po[:])
        pt = pst.tile([128, 4, H + 1], F32, tag="t")
        for t in range(4):
            nc.tensor.transpose(pt[:, t, :], ot[:, 128 * t:128 * (t + 1)],
                                idn[0:H + 1, 0:H + 1], )
        on = onat.tile([128, 4, H], F32, tag="on")
        for t in range(4):
            rc = onat.tile([128, 1], F32, tag="rc")
            nc.vector.reciprocal(rc[:], pt[:, t, H:H + 1])
            nc.scalar.activation(on[:, t, :], pt[:, t, 0:H], Copy,
                                 scale=rc[:, 0:1])
        dst = out_d[sl].rearrange("(t p) h -> p t h", p=128)
        nc.sync.dma_start(dst, on[:])


def build_nc(repeats=1, mm_dt=None):
    """Build the per-core Bass program (SPMD: same program on all 8 cores).

    repeats > 1 wraps the body in an on-device For_i loop; used only by the
    benchmarking harness to amortize host/launch overhead out of timing.
    """
    global MM_DT
    if mm_dt is not None:
        MM_DT = mm_dt
    _patch_tile_drain()
    nc = bass.Bass("TRN2", target_bir_lowering=False, debug=False)

    xt_d = nc.dram_tensor("xt", [NCC, 128, T], MM_DT, kind="ExternalInput")
    wa_d = nc.dram_tensor("wa", [NCC, 128, 128], MM_DT, kind="ExternalInput")
    wv_d = nc.dram_tensor("wv", [NCC, 128, H], MM_DT, kind="ExternalInput")
    tri_d = nc.dram_tensor("tri", [128, 128], F32, kind="ExternalInput")
    idn_d = nc.dram_tensor("idn", [128, 128], F32, kind="ExternalInput")
    out_d = nc.dram_tensor("out", [T, H], F32, kind="ExternalOutput")
    dram = (xt_d, wa_d, wv_d, tri_d, idn_d, out_d)

    with tile.TileContext(nc) as tc:
        with (
            tc.tile_pool(name="persist", bufs=1) as persist,
            tc.tile_pool(name="epool", bufs=3) as epool,
            tc.tile_pool(name="onat", bufs=2) as onat,
            tc.tile_pool(name="psw", bufs=2, space="PSUM") as psw,
            tc.tile_pool(name="pso", bufs=2, space="PSUM") as pso,
            tc.tile_pool(name="pst", bufs=2, space="PSUM") as pst,
        ):
            pools = (persist, epool, onat, psw, pso, pst)
            if repeats == 1:
                _attention_body(nc, tc, pools, dram)
            else:
                with tc.For_i(0, repeats, 1):
                    _attention_body(nc, tc, pools, dram)
    _split_excess_waits(nc)
    return nc


def make_in_maps(x, Wk, Wq, Wv):
    """Host-side layout prep: per-core transposed x, packed weights, masks."""
    x = np.asarray(x, dtype=np.float32)
    Wk = np.asarray(Wk, dtype=np.float32)
    Wq = np.asarray(Wq, dtype=np.float32)
    Wv = np.asarray(Wv, dtype=np.float32)

    wa = np.concatenate([Wq, Wk], axis=1).reshape(NCC, 128, 128)
    wv = Wv.reshape(NCC, 128, H)
    r = np.arange(128)
    tri = (r[:, None] <= r[None, :]).astype(np.float32)  # keep tk <= tq
    idn = np.eye(128, dtype=np.float32)
    common = {"wa": np.ascontiguousarray(wa), "wv": np.ascontiguousarray(wv),
              "tri": tri, "idn": idn}
    in_maps = []
    for b in range(B):
        xt = np.ascontiguousarray(x[b].T).reshape(NCC, 128, T)
        in_maps.append({"xt": xt, **common})
    return in_maps


def kernel(x, Wk, Wq, Wv):
    nc = build_nc(repeats=1)
    in_maps = make_in_maps(x, Wk, Wq, Wv)
    res = bass_utils.run_bass_kernel_spmd(nc, in_maps, core_ids=list(range(B)))
    return np.stack([res.results[b]["out"] for b in range(B)], axis=0)
